# revision 11
# baseline (speedup 1.0000x reference)
"""DiscreteWaveletUpsample Trainium2 kernel.

Math: out = conv3x3(haar_upsample(conv3x3(x, pre_w) + pre_b), post_w) + post_b

Device algorithm (per core, one batch sample, data-parallel over batch=8):
  * The fixed Haar reconstruction (stride-2 transposed conv with
    non-overlapping 2x2 taps) is folded into the pre-conv weights:
    Y(p,q)[c,h,w] (p,q in {0,1} polyphase components of the upsampled
    image y[c, 2h+p, 2w+q]) is itself a 3x3 conv of x with effective
    weights  Weff[p,q,c] = sum_s haar[s,p,q] * pre_w[s*64+c].
  * Stage 1: for p in {0,1}: conv producing [Y(p,0); Y(p,1)] stacked on
    128 partitions, via 9 tap-matmuls (K=cin=64) accumulated in PSUM,
    evacuated (bias added, cast bf16) into zero-padded SBUF images.
  * Stage 2: the post conv in polyphase space: each output component
    (p,q) is a sum of 6 K=128 matmuls over the stacked Y buffers
    (tap offsets of the polyphase-decomposed 3x3 kernel; weights zero
    where a tap does not contribute).  Evacuation adds post_b and
    interleaves components directly into full-resolution rows.
  * Full-res rows are DMA'd to HBM contiguously.
"""

import numpy as np
import ml_dtypes

import concourse.bass as bass
import concourse.mybir as mybir
import concourse.tile as tile
from concourse import bacc
from concourse.bass_utils import run_bass_kernel_spmd

N_CORES = 8
C = 64          # channels (cin = cout = 64; stage-1 produces 4*C subbands)
H = W = 128     # input spatial dims
HP, WP = H + 2, W + 2   # zero-padded
TAPS9 = [(ky, kx) for ky in range(3) for kx in range(3)]
COMPS = [(0, 0), (0, 1), (1, 0), (1, 1)]

F32 = mybir.dt.float32
BF16 = mybir.dt.bfloat16
NP_BF16 = ml_dtypes.bfloat16


# ----------------------------------------------------------------------------
# Host-side weight preparation
# ----------------------------------------------------------------------------

def _build_stage1_weights(pre_w, pre_b):
    """Fold the Haar reconstruction into the pre-conv weights.

    Returns
      w1[p, ky, kx, cin, m] float32, m = q*64 + c
      b1[m, p] float32
    """
    lo = np.array([0.5, 0.5], np.float32)
    hi = np.array([0.5, -0.5], np.float32)
    filt = np.stack([np.outer(lo, lo), np.outer(lo, hi),
                     np.outer(hi, lo), np.outer(hi, hi)], axis=0)  # [4,2,2]
    pw = pre_w.reshape(4, C, C, 3, 3).astype(np.float32)
    pb = pre_b.reshape(4, C).astype(np.float32)
    weff = np.einsum('spq,scikl->pqcikl', filt, pw)   # [p,q,c,cin,ky,kx]
    beff = np.einsum('spq,sc->pqc', filt, pb)         # [p,q,c]
    w1 = np.transpose(weff, (0, 4, 5, 3, 1, 2)).reshape(2, 3, 3, C, 2 * C)
    b1 = beff.reshape(2, 2 * C).T.copy()              # [m, p]
    return w1, b1


def _stage2_mm_list(p, q):
    """Six (p_in, dy, dx) matmul descriptors for output component (p,q)."""
    out = []
    for p_in in (0, 1):
        dys = []
        for ky in range(3):
            j = p + ky - 1
            if (j & 1) == p_in:
                dys.append((j - p_in) // 2)
        union_dx = set()
        for q_in in (0, 1):
            for kx in range(3):
                jx = q + kx - 1
                if (jx & 1) == q_in:
                    union_dx.add((jx - q_in) // 2)
        for dy in dys:
            for dx in sorted(union_dx):
                out.append((p_in, dy, dx))
    assert len(out) == 6
    return out


MM2 = {p * 2 + q: _stage2_mm_list(p, q) for p, q in COMPS}


def _build_stage2_weights(post_w):
    """w2[pq, i, row, cout] float32, row = q_in*64 + cin."""
    w2 = np.zeros((4, 6, 2 * C, C), np.float32)
    pwf = post_w.astype(np.float32)
    for p, q in COMPS:
        pq = p * 2 + q
        for i, (p_in, dy, dx) in enumerate(MM2[pq]):
            ky = 2 * dy + p_in - p + 1
            assert 0 <= ky <= 2
            for q_in in (0, 1):
                kx = 2 * dx + q_in - q + 1
                if 0 <= kx <= 2:
                    w2[pq, i, q_in * C:(q_in + 1) * C, :] = pwf[:, :, ky, kx].T
    return w2


# ----------------------------------------------------------------------------
# Device module
# ----------------------------------------------------------------------------

import os
PARTS = os.environ.get("KERNEL_PARTS", "all")  # debug bisection: s1 / s2 / all


def _build_module():
    nc = bacc.Bacc("TRN2", target_bir_lowering=False, debug=False,
                   num_devices=N_CORES)

    x_d = nc.dram_tensor("x", [C, H, W], BF16, kind="ExternalInput")
    w1_d = nc.dram_tensor("w1", [C, 18 * 128], BF16, kind="ExternalInput")
    b1_d = nc.dram_tensor("b1", [128, 2], F32, kind="ExternalInput")
    w2_d = nc.dram_tensor("w2", [128, 24 * C], BF16, kind="ExternalInput")
    b2_d = nc.dram_tensor("b2", [C, 1], F32, kind="ExternalInput")
    out_d = nc.dram_tensor("out", [C, 2 * H, 2 * W], F32,
                           kind="ExternalOutput")

    with tile.TileContext(nc) as tc:
        with (
            tc.tile_pool(name="const", bufs=1) as const,
            tc.tile_pool(name="xbuf", bufs=1) as xpool,
            tc.tile_pool(name="ybuf", bufs=1) as ypool,
            tc.tile_pool(name="psum", bufs=8, space="PSUM") as psum_pool,
            tc.tile_pool(name="stage", bufs=3) as stg,
        ):
            # ---- constants ----
            w1_s = const.tile([C, 18 * 128], BF16)
            nc.sync.dma_start(out=w1_s[:], in_=w1_d[:])
            w2_s = const.tile([128, 24 * C], BF16)
            nc.sync.dma_start(out=w2_s[:], in_=w2_d[:])
            b1_s = const.tile([128, 2], F32)
            nc.sync.dma_start(out=b1_s[:], in_=b1_d[:])
            b2_s = const.tile([C, 1], F32)
            nc.sync.dma_start(out=b2_s[:], in_=b2_d[:])

            # ---- input image, zero-padded ----
            xp = xpool.tile([C, HP, WP], BF16)
            nc.vector.memset(xp[:], 0.0)
            nc.sync.dma_start(out=xp[:, 1:H + 1, 1:W + 1], in_=x_d[:, :, :])

            # ---- Y buffers: buf[p] = [Y(p,0); Y(p,1)] stacked, padded ----
            ybufs = []
            for p in (0, 1):
                yb = ypool.tile([128, HP, WP], BF16, name=f"ybuf{p}")
                ybufs.append(yb)
                if PARTS == "s2":
                    nc.vector.memset(yb[:], 0.0)
                else:
                    # zero only the borders (interior written by stage 1)
                    nc.vector.memset(yb[:, 0, :], 0.0)
                    nc.vector.memset(yb[:, HP - 1, :], 0.0)
                    nc.vector.memset(yb[:, :, 0], 0.0)
                    nc.vector.memset(yb[:, :, WP - 1], 0.0)

            # ---- stage 1 ----
            for t in range(H // 4) if PARTS != "s2" else []:
                h0 = 4 * t
                for p in (0, 1):
                    ps = psum_pool.tile([128, 4, W], F32, name="psum_t", tag="ps")
                    for k, (ky, kx) in enumerate(TAPS9):
                        idx = (ky * 3 + kx) * 2 + p
                        nc.tensor.matmul(
                            ps[:, :, :],
                            w1_s[:, idx * 128:(idx + 1) * 128],
                            xp[:, h0 + ky:h0 + ky + 4, kx:kx + W],
                            start=(k == 0), stop=(k == 8),
                        )
                    nc.scalar.activation(
                        ybufs[p][:, h0 + 1:h0 + 5, 1:W + 1],
                        ps[:, :, :],
                        mybir.ActivationFunctionType.Identity,
                        bias=b1_s[:, p:p + 1],
                    )

            if PARTS == "barrier":
                tc.strict_bb_all_engine_barrier()

            if PARTS == "s1":
                # debug: dump a ybuf slice so stage-1 work is live
                nc.gpsimd.dma_start(out=out_d[:, 0:128, 0:130],
                                    in_=ybufs[0][0:64, 0:128, :])
                nc.gpsimd.dma_start(out=out_d[:, 128:256, 0:130],
                                    in_=ybufs[1][0:64, 0:128, :])

            # ---- stage 2 + interleave + store ----
            for t in range(H // 4) if PARTS != "s1" else []:
                h0 = 4 * t
                st = stg.tile([C, 8, 2 * W], F32, name="outstage")
                for p, q in COMPS:
                    pq = p * 2 + q
                    ps2 = psum_pool.tile([128, 4, W], F32, name="psum_t", tag="ps")[:C]
                    for i, (p_in, dy, dx) in enumerate(MM2[pq]):
                        idx = pq * 6 + i
                        nc.tensor.matmul(
                            ps2[:, :, :],
                            w2_s[:, idx * C:(idx + 1) * C],
                            ybufs[p_in][:, h0 + dy + 1:h0 + dy + 5,
                                        1 + dx:1 + dx + W],
                            start=(i == 0), stop=(i == 5),
                        )
                    nc.scalar.activation(
                        st[:, p::2, q::2],
                        ps2[:, :, :],
                        mybir.ActivationFunctionType.Identity,
                        bias=b2_s[:, 0:1],
                    )
                nc.sync.dma_start(out=out_d[:, 2 * h0:2 * h0 + 8, :],
                                  in_=st[:, :, :])

    nc.compile()
    return nc


_MODULE_CACHE = {}


def _get_module():
    if "nc" not in _MODULE_CACHE:
        _MODULE_CACHE["nc"] = _build_module()
    return _MODULE_CACHE["nc"]


# ----------------------------------------------------------------------------
# Entry point
# ----------------------------------------------------------------------------

def run(x, pre_w, pre_b, post_w, post_b, trace=False):
    x = np.asarray(x, np.float32)
    B = x.shape[0]
    assert B == N_CORES and x.shape == (B, C, H, W)

    w1, b1 = _build_stage1_weights(np.asarray(pre_w), np.asarray(pre_b))
    w2 = _build_stage2_weights(np.asarray(post_w))
    b2 = np.asarray(post_b, np.float32).reshape(C, 1)

    w1_flat = np.ascontiguousarray(
        np.transpose(w1, (3, 1, 2, 0, 4)).reshape(C, 18 * 128)
    ).astype(NP_BF16)
    # w1_flat[cin, ((ky*3+kx)*2+p)*128 + m] = w1[p, ky, kx, cin, m]
    w2_flat = np.ascontiguousarray(
        np.transpose(w2, (2, 0, 1, 3)).reshape(2 * C, 24 * C)
    ).astype(NP_BF16)
    # w2_flat[row, (pq*6+i)*64 + cout] = w2[pq, i, row, cout]

    b1_np = np.ascontiguousarray(b1, np.float32)       # [128, 2]
    x_bf = x.astype(NP_BF16)

    in_maps = []
    for b in range(B):
        in_maps.append({
            "x": np.ascontiguousarray(x_bf[b]),
            "w1": w1_flat,
            "b1": b1_np,
            "w2": w2_flat,
            "b2": b2,
        })

    nc = _get_module()
    res = run_bass_kernel_spmd(nc, in_maps, core_ids=list(range(N_CORES)),
                               trace=trace)
    out = np.stack([res.results[b]["out"] for b in range(B)])
    return out, res


def kernel(x, pre_w, pre_b, post_w, post_b):
    out, _ = run(x, pre_w, pre_b, post_w, post_b)
    return out


# revision 13
# speedup vs baseline: 1.4222x; 1.4222x over previous
"""DiscreteWaveletUpsample Trainium2 kernel.

Math: out = conv3x3(haar_upsample(conv3x3(x, pre_w) + pre_b), post_w) + post_b

Device algorithm (per core, one batch sample, data-parallel over batch=8):

  * The fixed Haar reconstruction (stride-2 transposed conv with
    non-overlapping 2x2 taps) is folded into the pre-conv weights:
    Y(p,q)[c,h,w] (the (p,q) polyphase components of the upsampled image,
    y[c, 2h+p, 2w+q] = Y(p,q)[c,h,w]) is itself a 3x3 conv of x with
    effective weights  Weff[p,q,c] = sum_s haar[s,p,q] * pre_w[s*64+c].

  * Stage 1 (per 4-row spatial tile, out ctile p): 9 tap-matmuls with
    K=cin=64 accumulate [Y(p,0); Y(p,1)] (M=128) in PSUM.  The PE runs in
    64x128 row-tiled mode: x is duplicated on both partition halves and
    tiles alternate row groups by spatial parity, so two tiles' matmul
    streams execute concurrently on the two sub-arrays (K=64 serial
    matmuls would otherwise run at half rate and never warm the HAM
    clock gate).  Evacuation (ScalarE/VectorE alternating) adds the bias
    and writes bf16 into zero-padded SBUF images.

  * The Y images are duplicated with partition halves swapped (SBUF to
    SBUF DMA) so that every polyphase component is available on both
    partition halves.

  * Stage 2 = the post conv in polyphase space: output component (p,q)
    at (h,w) sums 9 taps, each a K=64 matmul against component
    (p_in,q_in) at offset (dy,dx) (from the polyphase decomposition of
    the 3x3 kernel).  The PE runs in 64x64 four-tile mode: row group =
    component parity (q), column group = spatial-tile parity, so four
    matmul streams execute concurrently (full-array throughput at
    K=64/M=64, ~59 ns per 512-col matmul measured vs 223 serial).
    Evacuation adds post_b and interleaves components into
    full-resolution rows in SBUF staging; col-group-1 tiles land on
    partitions 64-127 and DMA out from there.

  * Full-res rows are DMA'd to HBM contiguously (512 KB per tile).
"""

import os

import numpy as np
import ml_dtypes

import concourse.bass as bass
import concourse.mybir as mybir
import concourse.tile as tile
from concourse import bacc
from concourse.bass_utils import run_bass_kernel_spmd

N_CORES = 8
C = 64          # channels (cin = cout = 64; stage-1 produces 4*C subbands)
H = W = 128     # input spatial dims
HP, WP = H + 2, W + 2   # zero-padded
TAPS9 = [(ky, kx) for ky in range(3) for kx in range(3)]
COMPS = [(0, 0), (0, 1), (1, 0), (1, 1)]

F32 = mybir.dt.float32
BF16 = mybir.dt.bfloat16
NP_BF16 = ml_dtypes.bfloat16

IDENT = mybir.ActivationFunctionType.Identity


# ----------------------------------------------------------------------------
# Host-side weight preparation
# ----------------------------------------------------------------------------

def _build_stage1_weights(pre_w, pre_b):
    """Fold the Haar reconstruction into the pre-conv weights.

    Returns
      w1[p, ky, kx, cin, m] float32, m = q*64 + c
      b1[m, p] float32
    """
    lo = np.array([0.5, 0.5], np.float32)
    hi = np.array([0.5, -0.5], np.float32)
    filt = np.stack([np.outer(lo, lo), np.outer(lo, hi),
                     np.outer(hi, lo), np.outer(hi, hi)], axis=0)  # [4,2,2]
    pw = pre_w.reshape(4, C, C, 3, 3).astype(np.float32)
    pb = pre_b.reshape(4, C).astype(np.float32)
    weff = np.einsum('spq,scikl->pqcikl', filt, pw)   # [p,q,c,cin,ky,kx]
    beff = np.einsum('spq,sc->pqc', filt, pb)         # [p,q,c]
    w1 = np.transpose(weff, (0, 4, 5, 3, 1, 2)).reshape(2, 3, 3, C, 2 * C)
    b1 = beff.reshape(2, 2 * C).T.copy()              # [m, p]
    return w1, b1


def _tap_decomp(p, q, ky, kx):
    """Polyphase decomposition of full-res tap (ky,kx) for out comp (p,q):
    returns (p_in, q_in, dy, dx)."""
    jy = p + ky - 1
    p_in = jy & 1
    dy = (jy - p_in) >> 1
    jx = q + kx - 1
    q_in = jx & 1
    dx = (jx - q_in) >> 1
    return p_in, q_in, dy, dx


def _build_stage2_weights(post_w):
    """w2[128, 36*64] bf16-ready float32.

    Column block (pq*9 + tap) holds lhsT [cin 64, cout 64] =
    post_w[:, :, ky, kx].T at partition rows [g*64:(g+1)*64] where
    g = q (the comp's row group); the other half is zero."""
    w2 = np.zeros((2 * C, 36 * C), np.float32)
    pwf = post_w.astype(np.float32)
    for p, q in COMPS:
        pq = p * 2 + q
        g = q
        for ti, (ky, kx) in enumerate(TAPS9):
            blk = (pq * 9 + ti) * C
            w2[g * C:(g + 1) * C, blk:blk + C] = pwf[:, :, ky, kx].T
    return w2


# ----------------------------------------------------------------------------
# Device module
# ----------------------------------------------------------------------------

PARTS = os.environ.get("KERNEL_PARTS", "all")  # debug bisection: s1 / s2 / all


def _build_module():
    nc = bacc.Bacc("TRN2", target_bir_lowering=False, debug=False,
                   num_devices=N_CORES)

    x_d = nc.dram_tensor("x", [C, H, W], BF16, kind="ExternalInput")
    w1_d = nc.dram_tensor("w1", [128, 18 * 128], BF16, kind="ExternalInput")
    b1_d = nc.dram_tensor("b1", [128, 2], F32, kind="ExternalInput")
    w2_d = nc.dram_tensor("w2", [128, 36 * C], BF16, kind="ExternalInput")
    b2_d = nc.dram_tensor("b2", [128, 1], F32, kind="ExternalInput")
    out_d = nc.dram_tensor("out", [C, 2 * H, 2 * W], F32,
                           kind="ExternalOutput")

    with tile.TileContext(nc) as tc:
        with (
            tc.tile_pool(name="const", bufs=1) as const,
            tc.tile_pool(name="xbuf", bufs=1) as xpool,
            tc.tile_pool(name="ybuf", bufs=1) as ypool,
            tc.tile_pool(name="psum", bufs=8, space="PSUM") as psum_pool,
            tc.tile_pool(name="stage", bufs=2) as stg,
        ):
            # ---- constants ----
            w1_s = const.tile([128, 18 * 128], BF16)
            nc.sync.dma_start(out=w1_s[:], in_=w1_d[:])
            w2_s = const.tile([128, 36 * C], BF16)
            nc.sync.dma_start(out=w2_s[:], in_=w2_d[:])
            b1_s = const.tile([128, 2], F32)
            nc.sync.dma_start(out=b1_s[:], in_=b1_d[:])
            b2_s = const.tile([128, 1], F32)
            nc.sync.dma_start(out=b2_s[:], in_=b2_d[:])

            # ---- input image, zero-padded, duplicated on both halves ----
            xp = xpool.tile([128, HP, WP], BF16)
            nc.vector.memset(xp[:], 0.0)
            nc.sync.dma_start(out=xp[0:C, 1:H + 1, 1:W + 1], in_=x_d[:, :, :])
            nc.sync.dma_start(out=xp[C:128, 1:H + 1, 1:W + 1], in_=x_d[:, :, :])

            # ---- Y buffers ----
            # ybufs[p][j]: partitions 0-63 = Y(p,j), 64-127 = Y(p,1-j);
            # j=0 written by stage-1 evac, j=1 is the partition-swapped DMA
            # copy.  Comp (p_in,q_in) on half g lives in ybufs[p_in][q_in^g].
            ybufs = [[None, None], [None, None]]
            for p in (0, 1):
                for j in (0, 1):
                    yb = ypool.tile([128, HP, WP], BF16, name=f"ybuf{p}{j}")
                    ybufs[p][j] = yb
                    if PARTS == "s2":
                        nc.vector.memset(yb[:], 0.0)
                        continue
                    nc.vector.memset(yb[:, 0, :], 0.0)
                    nc.vector.memset(yb[:, HP - 1, :], 0.0)
                    if j == 0:
                        # interior written by evac; dup copies full width
                        nc.vector.memset(yb[:, :, 0], 0.0)
                        nc.vector.memset(yb[:, :, WP - 1], 0.0)

            # ---- stage 1 ----
            for t in range(H // 4) if PARTS != "s2" else []:
                h0 = 4 * t
                g = t % 2
                gs = slice(g * C, (g + 1) * C)
                for p in (0, 1):
                    ps = psum_pool.tile([128, 4, W], F32, name="ps", tag="ps")
                    for k, (ky, kx) in enumerate(TAPS9):
                        idx = (ky * 3 + kx) * 2 + p
                        nc.tensor.matmul(
                            ps[:, :, :],
                            w1_s[gs, idx * 128:(idx + 1) * 128],
                            xp[gs, h0 + ky:h0 + ky + 4, kx:kx + W],
                            start=(k == 0), stop=(k == 8),
                            tile_position=(g * C, 0),
                        )
                    dst = ybufs[p][0][:, h0 + 1:h0 + 5, 1:W + 1]
                    if p == 0:
                        nc.scalar.activation(dst, ps[:, :, :], IDENT,
                                             bias=b1_s[:, p:p + 1])
                    else:
                        nc.vector.tensor_scalar_add(dst, ps[:, :, :],
                                                    b1_s[:, p:p + 1])
                    # duplicate with partition halves swapped
                    nc.sync.dma_start(
                        out=ybufs[p][1][0:C, h0 + 1:h0 + 5, :],
                        in_=ybufs[p][0][C:128, h0 + 1:h0 + 5, :])
                    nc.sync.dma_start(
                        out=ybufs[p][1][C:128, h0 + 1:h0 + 5, :],
                        in_=ybufs[p][0][0:C, h0 + 1:h0 + 5, :])

            if PARTS == "barrier":
                tc.strict_bb_all_engine_barrier()

            if PARTS == "s1":
                nc.gpsimd.dma_start(out=out_d[:, 0:128, 0:130],
                                    in_=ybufs[0][0][0:C, 0:128, :])
                nc.gpsimd.dma_start(out=out_d[:, 128:256, 0:130],
                                    in_=ybufs[1][0][0:C, 0:128, :])

            # ---- stage 2 + interleave + store ----
            for j in range(H // 8) if PARTS != "s1" else []:
                st = stg.tile([128, 8, 2 * W], F32, name="st", tag="st")
                for c in (0, 1):          # column group = spatial parity
                    t = 2 * j + c
                    h0 = 4 * t
                    cs = slice(c * C, (c + 1) * C)
                    for p, q in COMPS:
                        pq = p * 2 + q
                        g = q             # row group
                        gs = slice(g * C, (g + 1) * C)
                        ps = psum_pool.tile([128, 4, W], F32, name="ps",
                                            tag="ps")
                        for i, (ky, kx) in enumerate(TAPS9):
                            p_in, q_in, dy, dx = _tap_decomp(p, q, ky, kx)
                            rhs = ybufs[p_in][q_in ^ g][
                                gs, h0 + dy + 1:h0 + dy + 5,
                                1 + dx:1 + dx + W]
                            blk = (pq * 9 + i) * C
                            nc.tensor.matmul(
                                ps[cs, :, :],
                                w2_s[gs, blk:blk + C],
                                rhs,
                                start=(i == 0), stop=(i == 8),
                                tile_position=(g * C, c * C),
                            )
                        dst = st[cs, p::2, q::2]
                        if c == 0:
                            nc.scalar.activation(dst, ps[cs, :, :], IDENT,
                                                 bias=b2_s[cs, 0:1])
                        else:
                            nc.vector.tensor_scalar_add(dst, ps[cs, :, :],
                                                        b2_s[cs, 0:1])
                    nc.sync.dma_start(out=out_d[:, 8 * t:8 * t + 8, :],
                                      in_=st[cs, :, :])

    nc.compile()
    return nc


_MODULE_CACHE = {}


def _get_module():
    if "nc" not in _MODULE_CACHE:
        _MODULE_CACHE["nc"] = _build_module()
    return _MODULE_CACHE["nc"]


# ----------------------------------------------------------------------------
# Entry point
# ----------------------------------------------------------------------------

def prep_weight_map(pre_w, pre_b, post_w, post_b):
    """Device-layout weight arrays, shared across cores."""
    w1, b1 = _build_stage1_weights(np.asarray(pre_w), np.asarray(pre_b))
    w2 = _build_stage2_weights(np.asarray(post_w))
    b2 = np.asarray(post_b, np.float32).reshape(C, 1)

    w1_half = np.transpose(w1, (3, 1, 2, 0, 4)).reshape(C, 18 * 128)
    # w1_half[cin, ((ky*3+kx)*2+p)*128 + m] = w1[p, ky, kx, cin, m]
    w1_flat = np.ascontiguousarray(
        np.concatenate([w1_half, w1_half], axis=0)).astype(NP_BF16)
    w2_flat = np.ascontiguousarray(w2).astype(NP_BF16)
    return {
        "w1": w1_flat,
        "b1": np.ascontiguousarray(b1, np.float32),                # [128, 2]
        "w2": w2_flat,
        "b2": np.ascontiguousarray(np.vstack([b2, b2]), np.float32),
    }


def run(x, pre_w, pre_b, post_w, post_b, trace=False):
    x = np.asarray(x, np.float32)
    B = x.shape[0]
    assert B == N_CORES and x.shape == (B, C, H, W)

    wmap = prep_weight_map(pre_w, pre_b, post_w, post_b)
    x_bf = x.astype(NP_BF16)

    in_maps = []
    for b in range(B):
        in_maps.append({"x": np.ascontiguousarray(x_bf[b]), **wmap})

    nc = _get_module()
    res = run_bass_kernel_spmd(nc, in_maps, core_ids=list(range(N_CORES)),
                               trace=trace)
    out = np.stack([res.results[b]["out"] for b in range(B)])
    return out, res


def kernel(x, pre_w, pre_b, post_w, post_b):
    out, _ = run(x, pre_w, pre_b, post_w, post_b)
    return out


# revision 15
# speedup vs baseline: 1.6803x; 1.1814x over previous
"""DiscreteWaveletUpsample Trainium2 kernel.

Math: out = conv3x3(haar_upsample(conv3x3(x, pre_w) + pre_b), post_w) + post_b

Device algorithm (per core, one batch sample, data-parallel over batch=8):

  * The fixed Haar reconstruction (stride-2 transposed conv with
    non-overlapping 2x2 taps) is folded into the pre-conv weights:
    Y(p,q)[c,h,w] (the (p,q) polyphase components of the upsampled image,
    y[c, 2h+p, 2w+q] = Y(p,q)[c,h,w]) is itself a 3x3 conv of x with
    effective weights  Weff[p,q,c] = sum_s haar[s,p,q] * pre_w[s*64+c].

  * Stage 1 (per 4-row spatial tile, out ctile p): 9 tap-matmuls with
    K=cin=64 accumulate [Y(p,0); Y(p,1)] (M=128) in PSUM.  The PE runs in
    64x128 row-tiled mode: x is duplicated on both partition halves and
    tiles alternate row groups by spatial parity, so two tiles' matmul
    streams execute concurrently on the two sub-arrays (K=64 serial
    matmuls would otherwise run at half rate and never warm the HAM
    clock gate).  Evacuation (ScalarE/VectorE alternating) adds the bias
    and writes bf16 into zero-padded SBUF images.

  * The Y images are duplicated with partition halves swapped (SBUF to
    SBUF DMA) so that every polyphase component is available on both
    partition halves.

  * Stage 2 = the post conv in polyphase space: output component (p,q)
    at (h,w) sums 9 taps, each a K=64 matmul against component
    (p_in,q_in) at offset (dy,dx) (from the polyphase decomposition of
    the 3x3 kernel).  The PE runs in 64x64 four-tile mode: row group =
    component parity (q), column group = spatial-tile parity, so four
    matmul streams execute concurrently (full-array throughput at
    K=64/M=64, ~59 ns per 512-col matmul measured vs 223 serial).
    Evacuation adds post_b and interleaves components into
    full-resolution rows in SBUF staging; col-group-1 tiles land on
    partitions 64-127 and DMA out from there.

  * Full-res rows are DMA'd to HBM contiguously (512 KB per tile).
"""

import os

import numpy as np
import ml_dtypes

import concourse.bass as bass
import concourse.mybir as mybir
import concourse.tile as tile
from concourse import bacc
from concourse.bass_utils import run_bass_kernel_spmd

N_CORES = 8
C = 64          # channels (cin = cout = 64; stage-1 produces 4*C subbands)
H = W = 128     # input spatial dims
HP, WP = H + 2, W + 2   # zero-padded
TAPS9 = [(ky, kx) for ky in range(3) for kx in range(3)]
COMPS = [(0, 0), (0, 1), (1, 0), (1, 1)]

F32 = mybir.dt.float32
BF16 = mybir.dt.bfloat16
NP_BF16 = ml_dtypes.bfloat16

IDENT = mybir.ActivationFunctionType.Identity


# ----------------------------------------------------------------------------
# Host-side weight preparation
# ----------------------------------------------------------------------------

def _build_stage1_weights(pre_w, pre_b):
    """Fold the Haar reconstruction into the pre-conv weights.

    Returns
      w1[p, ky, kx, cin, m] float32, m = q*64 + c
      b1[m, p] float32
    """
    lo = np.array([0.5, 0.5], np.float32)
    hi = np.array([0.5, -0.5], np.float32)
    filt = np.stack([np.outer(lo, lo), np.outer(lo, hi),
                     np.outer(hi, lo), np.outer(hi, hi)], axis=0)  # [4,2,2]
    pw = pre_w.reshape(4, C, C, 3, 3).astype(np.float32)
    pb = pre_b.reshape(4, C).astype(np.float32)
    weff = np.einsum('spq,scikl->pqcikl', filt, pw)   # [p,q,c,cin,ky,kx]
    beff = np.einsum('spq,sc->pqc', filt, pb)         # [p,q,c]
    w1 = np.transpose(weff, (0, 4, 5, 3, 1, 2)).reshape(2, 3, 3, C, 2 * C)
    b1 = beff.reshape(2, 2 * C).T.copy()              # [m, p]
    return w1, b1


def _tap_decomp(p, q, ky, kx):
    """Polyphase decomposition of full-res tap (ky,kx) for out comp (p,q):
    returns (p_in, q_in, dy, dx)."""
    jy = p + ky - 1
    p_in = jy & 1
    dy = (jy - p_in) >> 1
    jx = q + kx - 1
    q_in = jx & 1
    dx = (jx - q_in) >> 1
    return p_in, q_in, dy, dx


def _build_stage2_weights(post_w):
    """w2[128, 36*64] bf16-ready float32.

    Column block (pq*9 + tap) holds lhsT [cin 64, cout 64] =
    post_w[:, :, ky, kx].T at partition rows [g*64:(g+1)*64] where
    g = q (the comp's row group); the other half is zero."""
    w2 = np.zeros((2 * C, 36 * C), np.float32)
    pwf = post_w.astype(np.float32)
    for p, q in COMPS:
        pq = p * 2 + q
        g = q
        for ti, (ky, kx) in enumerate(TAPS9):
            blk = (pq * 9 + ti) * C
            w2[g * C:(g + 1) * C, blk:blk + C] = pwf[:, :, ky, kx].T
    return w2


# ----------------------------------------------------------------------------
# Device module
# ----------------------------------------------------------------------------

PARTS = os.environ.get("KERNEL_PARTS", "all")  # debug bisection: s1 / s2 / all


def _build_module():
    nc = bacc.Bacc("TRN2", target_bir_lowering=False, debug=False,
                   num_devices=N_CORES)

    x_d = nc.dram_tensor("x", [C, H, W], BF16, kind="ExternalInput")
    w1_d = nc.dram_tensor("w1", [128, 18 * 128], BF16, kind="ExternalInput")
    b1_d = nc.dram_tensor("b1", [128, 2], F32, kind="ExternalInput")
    w2_d = nc.dram_tensor("w2", [128, 36 * C], BF16, kind="ExternalInput")
    b2_d = nc.dram_tensor("b2", [128, 1], F32, kind="ExternalInput")
    out_d = nc.dram_tensor("out", [C, 2 * H, 2 * W], F32,
                           kind="ExternalOutput")

    with tile.TileContext(nc) as tc:
        with (
            tc.tile_pool(name="const", bufs=1) as const,
            tc.tile_pool(name="xbuf", bufs=1) as xpool,
            tc.tile_pool(name="ybuf", bufs=1) as ypool,
            tc.tile_pool(name="psum", bufs=8, space="PSUM") as psum_pool,
            tc.tile_pool(name="stage", bufs=2) as stg,
        ):
            # ---- constants ----
            w1_s = const.tile([128, 18 * 128], BF16)
            nc.sync.dma_start(out=w1_s[:], in_=w1_d[:])
            w2_s = const.tile([128, 36 * C], BF16)
            nc.sync.dma_start(out=w2_s[:], in_=w2_d[:])
            b1_s = const.tile([128, 2], F32)
            nc.sync.dma_start(out=b1_s[:], in_=b1_d[:])
            b2_s = const.tile([128, 1], F32)
            nc.sync.dma_start(out=b2_s[:], in_=b2_d[:])

            # ---- input image, zero-padded, duplicated on both halves ----
            xp = xpool.tile([128, HP, WP], BF16)
            nc.vector.memset(xp[:], 0.0)
            nc.sync.dma_start(out=xp[0:C, 1:H + 1, 1:W + 1], in_=x_d[:, :, :])
            nc.sync.dma_start(out=xp[C:128, 1:H + 1, 1:W + 1], in_=x_d[:, :, :])

            # ---- Y buffers ----
            # ybufs[p][j]: partitions 0-63 = Y(p,j), 64-127 = Y(p,1-j);
            # j=0 written by stage-1 evac, j=1 is the partition-swapped DMA
            # copy.  Comp (p_in,q_in) on half g lives in ybufs[p_in][q_in^g].
            ybufs = [[None, None], [None, None]]
            for p in (0, 1):
                for j in (0, 1):
                    yb = ypool.tile([128, HP, WP], BF16, name=f"ybuf{p}{j}")
                    ybufs[p][j] = yb
                    if PARTS == "s2":
                        nc.vector.memset(yb[:], 0.0)
                        continue
                    nc.vector.memset(yb[:, 0, :], 0.0)
                    nc.vector.memset(yb[:, HP - 1, :], 0.0)
                    if j == 0:
                        # interior written by evac; dup copies full width
                        nc.vector.memset(yb[:, :, 0], 0.0)
                        nc.vector.memset(yb[:, :, WP - 1], 0.0)

            # ---- stage 1 ----
            # Supers of 8 spatial tiles, split by out-ctile p: the 8 PSUM
            # banks hold one p-phase of a super; taps are outermost so each
            # tap's weights are loaded once per row group and reused for 4
            # matmuls (the 4 same-parity tiles of the super).
            for sup in range(H // 32) if PARTS != "s2" else []:
                ts_all = list(range(8 * sup, 8 * sup + 8))
                for p in (0, 1):
                    accs = {}
                    for t in ts_all:
                        accs[t] = psum_pool.tile([128, 4, W], F32,
                                                 name="ps", tag="ps")
                    for k, (ky, kx) in enumerate(TAPS9):
                        idx = (ky * 3 + kx) * 2 + p
                        for g in (0, 1):
                            gs = slice(g * C, (g + 1) * C)
                            for t in ts_all[g::2]:
                                h0 = 4 * t
                                nc.tensor.matmul(
                                    accs[t][:, :, :],
                                    w1_s[gs, idx * 128:(idx + 1) * 128],
                                    xp[gs, h0 + ky:h0 + ky + 4, kx:kx + W],
                                    start=(k == 0), stop=(k == 8),
                                    tile_position=(g * C, 0),
                                )
                    for t in ts_all:
                        h0 = 4 * t
                        dst = ybufs[p][0][:, h0 + 1:h0 + 5, 1:W + 1]
                        if t % 2 == 0:
                            nc.scalar.activation(dst, accs[t][:, :, :], IDENT,
                                                 bias=b1_s[:, p:p + 1])
                        else:
                            nc.vector.tensor_scalar_add(dst, accs[t][:, :, :],
                                                        b1_s[:, p:p + 1])
                        # duplicate with partition halves swapped
                        nc.sync.dma_start(
                            out=ybufs[p][1][0:C, h0 + 1:h0 + 5, :],
                            in_=ybufs[p][0][C:128, h0 + 1:h0 + 5, :])
                        nc.sync.dma_start(
                            out=ybufs[p][1][C:128, h0 + 1:h0 + 5, :],
                            in_=ybufs[p][0][0:C, h0 + 1:h0 + 5, :])

            if PARTS == "barrier":
                tc.strict_bb_all_engine_barrier()

            if PARTS == "s1":
                nc.gpsimd.dma_start(out=out_d[:, 0:128, 0:130],
                                    in_=ybufs[0][0][0:C, 0:128, :])
                nc.gpsimd.dma_start(out=out_d[:, 128:256, 0:130],
                                    in_=ybufs[1][0][0:C, 0:128, :])

            # ---- stage 2 + interleave + store ----
            # Four-tile mode: row group g = comp q, col group c = spatial
            # parity.  Taps outermost: per tap each sub-array loads its
            # weights once and runs the two p-matmuls (same weights -- the
            # stage-2 weight depends only on (ky,kx)).
            for j in range(H // 8) if PARTS != "s1" else []:
                st = stg.tile([128, 8, 2 * W], F32, name="st", tag="st")
                accs = {}
                for c in (0, 1):
                    for p, q in COMPS:
                        accs[c, p, q] = psum_pool.tile([128, 4, W], F32,
                                                       name="ps", tag="ps")
                for i, (ky, kx) in enumerate(TAPS9):
                    for c in (0, 1):
                        t = 2 * j + c
                        h0 = 4 * t
                        cs = slice(c * C, (c + 1) * C)
                        for q in (0, 1):
                            g = q
                            gs = slice(g * C, (g + 1) * C)
                            blk = ((q * 9) + i) * C   # pq=q block (p=0)
                            for p in (0, 1):
                                pq = p * 2 + q
                                blk = (pq * 9 + i) * C
                                p_in, q_in, dy, dx = _tap_decomp(p, q, ky, kx)
                                rhs = ybufs[p_in][q_in ^ g][
                                    gs, h0 + dy + 1:h0 + dy + 5,
                                    1 + dx:1 + dx + W]
                                nc.tensor.matmul(
                                    accs[c, p, q][cs, :, :],
                                    w2_s[gs, blk:blk + C],
                                    rhs,
                                    start=(i == 0), stop=(i == 8),
                                    tile_position=(g * C, c * C),
                                )
                for c in (0, 1):
                    t = 2 * j + c
                    cs = slice(c * C, (c + 1) * C)
                    for p, q in COMPS:
                        dst = st[cs, p::2, q::2]
                        if c == 0:
                            nc.scalar.activation(dst, accs[c, p, q][cs, :, :],
                                                 IDENT, bias=b2_s[cs, 0:1])
                        else:
                            nc.vector.tensor_scalar_add(
                                dst, accs[c, p, q][cs, :, :], b2_s[cs, 0:1])
                    nc.sync.dma_start(out=out_d[:, 8 * t:8 * t + 8, :],
                                      in_=st[cs, :, :])

    nc.compile()
    return nc


_MODULE_CACHE = {}


def _get_module():
    if "nc" not in _MODULE_CACHE:
        _MODULE_CACHE["nc"] = _build_module()
    return _MODULE_CACHE["nc"]


# ----------------------------------------------------------------------------
# Entry point
# ----------------------------------------------------------------------------

def prep_weight_map(pre_w, pre_b, post_w, post_b):
    """Device-layout weight arrays, shared across cores."""
    w1, b1 = _build_stage1_weights(np.asarray(pre_w), np.asarray(pre_b))
    w2 = _build_stage2_weights(np.asarray(post_w))
    b2 = np.asarray(post_b, np.float32).reshape(C, 1)

    w1_half = np.transpose(w1, (3, 1, 2, 0, 4)).reshape(C, 18 * 128)
    # w1_half[cin, ((ky*3+kx)*2+p)*128 + m] = w1[p, ky, kx, cin, m]
    w1_flat = np.ascontiguousarray(
        np.concatenate([w1_half, w1_half], axis=0)).astype(NP_BF16)
    w2_flat = np.ascontiguousarray(w2).astype(NP_BF16)
    return {
        "w1": w1_flat,
        "b1": np.ascontiguousarray(b1, np.float32),                # [128, 2]
        "w2": w2_flat,
        "b2": np.ascontiguousarray(np.vstack([b2, b2]), np.float32),
    }


def run(x, pre_w, pre_b, post_w, post_b, trace=False):
    x = np.asarray(x, np.float32)
    B = x.shape[0]
    assert B == N_CORES and x.shape == (B, C, H, W)

    wmap = prep_weight_map(pre_w, pre_b, post_w, post_b)
    x_bf = x.astype(NP_BF16)

    in_maps = []
    for b in range(B):
        in_maps.append({"x": np.ascontiguousarray(x_bf[b]), **wmap})

    nc = _get_module()
    res = run_bass_kernel_spmd(nc, in_maps, core_ids=list(range(N_CORES)),
                               trace=trace)
    out = np.stack([res.results[b]["out"] for b in range(B)])
    return out, res


def kernel(x, pre_w, pre_b, post_w, post_b):
    out, _ = run(x, pre_w, pre_b, post_w, post_b)
    return out


# revision 16
# speedup vs baseline: 1.6997x; 1.0116x over previous
"""DiscreteWaveletUpsample Trainium2 kernel.

Math: out = conv3x3(haar_upsample(conv3x3(x, pre_w) + pre_b), post_w) + post_b

Device algorithm (per core, one batch sample, data-parallel over batch=8):

  * The fixed Haar reconstruction (stride-2 transposed conv with
    non-overlapping 2x2 taps) is folded into the pre-conv weights:
    Y(p,q)[c,h,w] (the (p,q) polyphase components of the upsampled image,
    y[c, 2h+p, 2w+q] = Y(p,q)[c,h,w]) is itself a 3x3 conv of x with
    effective weights  Weff[p,q,c] = sum_s haar[s,p,q] * pre_w[s*64+c].

  * Stage 1 (per 4-row spatial tile, out ctile p): 9 tap-matmuls with
    K=cin=64 accumulate [Y(p,0); Y(p,1)] (M=128) in PSUM.  The PE runs in
    64x128 row-tiled mode: x is duplicated on both partition halves and
    tiles alternate row groups by spatial parity, so two tiles' matmul
    streams execute concurrently on the two sub-arrays (K=64 serial
    matmuls would otherwise run at half rate and never warm the HAM
    clock gate).  Evacuation (ScalarE/VectorE alternating) adds the bias
    and writes bf16 into zero-padded SBUF images.

  * The Y images are duplicated with partition halves swapped (SBUF to
    SBUF DMA) so that every polyphase component is available on both
    partition halves.

  * Stage 2 = the post conv in polyphase space: output component (p,q)
    at (h,w) sums 9 taps, each a K=64 matmul against component
    (p_in,q_in) at offset (dy,dx) (from the polyphase decomposition of
    the 3x3 kernel).  The PE runs in 64x64 four-tile mode: row group =
    component parity (q), column group = spatial-tile parity, so four
    matmul streams execute concurrently (full-array throughput at
    K=64/M=64, ~59 ns per 512-col matmul measured vs 223 serial).
    Evacuation adds post_b and interleaves components into
    full-resolution rows in SBUF staging; col-group-1 tiles land on
    partitions 64-127 and DMA out from there.

  * Full-res rows are DMA'd to HBM contiguously (512 KB per tile).
"""

import os

import numpy as np
import ml_dtypes

import concourse.bass as bass
import concourse.mybir as mybir
import concourse.tile as tile
from concourse import bacc
from concourse.tile_rust import add_dep_helper
from concourse.bass_utils import run_bass_kernel_spmd

N_CORES = 8
C = 64          # channels (cin = cout = 64; stage-1 produces 4*C subbands)
H = W = 128     # input spatial dims
HP, WP = H + 2, W + 2   # zero-padded
TAPS9 = [(ky, kx) for ky in range(3) for kx in range(3)]
COMPS = [(0, 0), (0, 1), (1, 0), (1, 1)]

F32 = mybir.dt.float32
BF16 = mybir.dt.bfloat16
NP_BF16 = ml_dtypes.bfloat16

IDENT = mybir.ActivationFunctionType.Identity


# ----------------------------------------------------------------------------
# Host-side weight preparation
# ----------------------------------------------------------------------------

def _build_stage1_weights(pre_w, pre_b):
    """Fold the Haar reconstruction into the pre-conv weights.

    Returns
      w1[p, ky, kx, cin, m] float32, m = q*64 + c
      b1[m, p] float32
    """
    lo = np.array([0.5, 0.5], np.float32)
    hi = np.array([0.5, -0.5], np.float32)
    filt = np.stack([np.outer(lo, lo), np.outer(lo, hi),
                     np.outer(hi, lo), np.outer(hi, hi)], axis=0)  # [4,2,2]
    pw = pre_w.reshape(4, C, C, 3, 3).astype(np.float32)
    pb = pre_b.reshape(4, C).astype(np.float32)
    weff = np.einsum('spq,scikl->pqcikl', filt, pw)   # [p,q,c,cin,ky,kx]
    beff = np.einsum('spq,sc->pqc', filt, pb)         # [p,q,c]
    w1 = np.transpose(weff, (0, 4, 5, 3, 1, 2)).reshape(2, 3, 3, C, 2 * C)
    b1 = beff.reshape(2, 2 * C).T.copy()              # [m, p]
    return w1, b1


def _tap_decomp(p, q, ky, kx):
    """Polyphase decomposition of full-res tap (ky,kx) for out comp (p,q):
    returns (p_in, q_in, dy, dx)."""
    jy = p + ky - 1
    p_in = jy & 1
    dy = (jy - p_in) >> 1
    jx = q + kx - 1
    q_in = jx & 1
    dx = (jx - q_in) >> 1
    return p_in, q_in, dy, dx


def _build_stage2_weights(post_w):
    """w2[128, 36*64] bf16-ready float32.

    Column block (pq*9 + tap) holds lhsT [cin 64, cout 64] =
    post_w[:, :, ky, kx].T at partition rows [g*64:(g+1)*64] where
    g = q (the comp's row group); the other half is zero."""
    w2 = np.zeros((2 * C, 36 * C), np.float32)
    pwf = post_w.astype(np.float32)
    for p, q in COMPS:
        pq = p * 2 + q
        g = q
        for ti, (ky, kx) in enumerate(TAPS9):
            blk = (pq * 9 + ti) * C
            w2[g * C:(g + 1) * C, blk:blk + C] = pwf[:, :, ky, kx].T
    return w2


# ----------------------------------------------------------------------------
# Device module
# ----------------------------------------------------------------------------

PARTS = os.environ.get("KERNEL_PARTS", "all")  # debug bisection: s1 / s2 / all


def _build_module():
    nc = bacc.Bacc("TRN2", target_bir_lowering=False, debug=False,
                   num_devices=N_CORES)

    x_d = nc.dram_tensor("x", [C, H, W], BF16, kind="ExternalInput")
    w1_d = nc.dram_tensor("w1", [128, 18 * 128], BF16, kind="ExternalInput")
    b1_d = nc.dram_tensor("b1", [128, 2], F32, kind="ExternalInput")
    w2_d = nc.dram_tensor("w2", [128, 36 * C], BF16, kind="ExternalInput")
    b2_d = nc.dram_tensor("b2", [128, 1], F32, kind="ExternalInput")
    out_d = nc.dram_tensor("out", [C, 2 * H, 2 * W], F32,
                           kind="ExternalOutput")

    with tile.TileContext(nc) as tc:
        with (
            tc.tile_pool(name="const", bufs=1) as const,
            tc.tile_pool(name="xbuf", bufs=1) as xpool,
            tc.tile_pool(name="ybuf", bufs=1) as ypool,
            tc.tile_pool(name="psum", bufs=8, space="PSUM") as psum_pool,
            tc.tile_pool(name="stage", bufs=2) as stg,
        ):
            # ---- constants ----
            w1_s = const.tile([128, 18 * 128], BF16)
            nc.sync.dma_start(out=w1_s[:], in_=w1_d[:])
            w2_s = const.tile([128, 36 * C], BF16)
            nc.sync.dma_start(out=w2_s[:], in_=w2_d[:])
            b1_s = const.tile([128, 2], F32)
            nc.sync.dma_start(out=b1_s[:], in_=b1_d[:])
            b2_s = const.tile([128, 1], F32)
            nc.sync.dma_start(out=b2_s[:], in_=b2_d[:])

            # ---- input image, zero-padded, duplicated on both halves ----
            xp = xpool.tile([128, HP, WP], BF16)
            nc.gpsimd.memset(xp[:], 0.0)
            nc.sync.dma_start(out=xp[0:C, 1:H + 1, 1:W + 1], in_=x_d[:, :, :])
            nc.sync.dma_start(out=xp[C:128, 1:H + 1, 1:W + 1], in_=x_d[:, :, :])

            # ---- Y buffers ----
            # ybufs[p][j]: partitions 0-63 = Y(p,j), 64-127 = Y(p,1-j);
            # j=0 written by stage-1 evac, j=1 is the partition-swapped DMA
            # copy.  Comp (p_in,q_in) on half g lives in ybufs[p_in][q_in^g].
            ybufs = [[None, None], [None, None]]
            for p in (0, 1):
                for j in (0, 1):
                    yb = ypool.tile([128, HP, WP], BF16, name=f"ybuf{p}{j}")
                    ybufs[p][j] = yb
                    if PARTS == "s2":
                        nc.gpsimd.memset(yb[:], 0.0)
                        continue
                    nc.gpsimd.memset(yb[:, 0, :], 0.0)
                    nc.gpsimd.memset(yb[:, HP - 1, :], 0.0)
                    if j == 0:
                        # interior written by evac; dup copies full width
                        nc.gpsimd.memset(yb[:, :, 0], 0.0)
                        nc.gpsimd.memset(yb[:, :, WP - 1], 0.0)

            # ---- matmul emission: per-sub-array order chain + LDW elision --
            last_mm = {}
            last_w = {}

            def mm(out_ap, w_ap, w_key, rhs_ap, start, stop, pos):
                inst = nc.tensor.matmul(out_ap, w_ap, rhs_ap,
                                        start=start, stop=stop,
                                        tile_position=pos)
                prev = last_mm.get(pos)
                if prev is not None:
                    add_dep_helper(inst.ins, prev, sync=False,
                                   reason="pe-subarray-order")
                    if last_w.get(pos) == w_key:
                        inst.ins.ldweights = False
                last_mm[pos] = inst.ins
                last_w[pos] = w_key

            def stage1_super(sup):
                # Supers of 8 spatial tiles, split by out-ctile p: the 8 PSUM
                # banks hold one p-phase; taps outermost so each tap's
                # weights load once per row group, reused for 4 matmuls.
                ts_all = list(range(8 * sup, 8 * sup + 8))
                for p in (0, 1):
                    accs = {}
                    for t in ts_all:
                        accs[t] = psum_pool.tile([128, 4, W], F32,
                                                 name="ps", tag="ps")
                    for k, (ky, kx) in enumerate(TAPS9):
                        idx = (ky * 3 + kx) * 2 + p
                        for g in (0, 1):
                            gs = slice(g * C, (g + 1) * C)
                            for t in ts_all[g::2]:
                                h0 = 4 * t
                                mm(accs[t][:, :, :],
                                   w1_s[gs, idx * 128:(idx + 1) * 128],
                                   ("s1", idx),
                                   xp[gs, h0 + ky:h0 + ky + 4, kx:kx + W],
                                   k == 0, k == 8, (g * C, 0))
                    for t in ts_all:
                        h0 = 4 * t
                        dst = ybufs[p][0][:, h0 + 1:h0 + 5, 1:W + 1]
                        if t % 2 == 0:
                            nc.scalar.activation(dst, accs[t][:, :, :], IDENT,
                                                 bias=b1_s[:, p:p + 1])
                        else:
                            nc.vector.tensor_scalar_add(dst, accs[t][:, :, :],
                                                        b1_s[:, p:p + 1])
                        # duplicate with partition halves swapped
                        nc.sync.dma_start(
                            out=ybufs[p][1][0:C, h0 + 1:h0 + 5, :],
                            in_=ybufs[p][0][C:128, h0 + 1:h0 + 5, :])
                        nc.sync.dma_start(
                            out=ybufs[p][1][C:128, h0 + 1:h0 + 5, :],
                            in_=ybufs[p][0][0:C, h0 + 1:h0 + 5, :])

            def stage2_block(j):
                # Four-tile mode: row group g = comp q, col group c = spatial
                # parity.  Taps outermost; the two p-matmuls per sub-array
                # share the tap's weights (stage-2 weights depend only on
                # (ky,kx)), so the second skips its LDWEIGHTS.
                st = stg.tile([128, 8, 2 * W], F32, name="st", tag="st")
                accs = {}
                for c in (0, 1):
                    for p, q in COMPS:
                        accs[c, p, q] = psum_pool.tile([128, 4, W], F32,
                                                       name="ps", tag="ps")
                for i, (ky, kx) in enumerate(TAPS9):
                    for c in (0, 1):
                        t = 2 * j + c
                        h0 = 4 * t
                        cs = slice(c * C, (c + 1) * C)
                        for q in (0, 1):
                            g = q
                            gs = slice(g * C, (g + 1) * C)
                            for p in (0, 1):
                                pq = p * 2 + q
                                blk = (pq * 9 + i) * C
                                p_in, q_in, dy, dx = _tap_decomp(p, q, ky, kx)
                                rhs = ybufs[p_in][q_in ^ g][
                                    gs, h0 + dy + 1:h0 + dy + 5,
                                    1 + dx:1 + dx + W]
                                mm(accs[c, p, q][cs, :, :],
                                   w2_s[gs, blk:blk + C],
                                   ("s2", ky, kx),
                                   rhs, i == 0, i == 8, (g * C, c * C))
                for c in (0, 1):
                    t = 2 * j + c
                    cs = slice(c * C, (c + 1) * C)
                    for p, q in COMPS:
                        dst = st[cs, p::2, q::2]
                        if c == 0:
                            nc.scalar.activation(dst, accs[c, p, q][cs, :, :],
                                                 IDENT, bias=b2_s[cs, 0:1])
                        else:
                            nc.vector.tensor_scalar_add(
                                dst, accs[c, p, q][cs, :, :], b2_s[cs, 0:1])
                    nc.sync.dma_start(out=out_d[:, 8 * t:8 * t + 8, :],
                                      in_=st[cs, :, :])

            # ---- interleaved emission: stage-2 block j needs stage-1 tiles
            # through 2j+2, i.e. supers through ceil((2j+2-7)/8) ----
            n_sup = H // 32
            if PARTS == "s1":
                for sup in range(n_sup):
                    stage1_super(sup)
                nc.gpsimd.dma_start(out=out_d[:, 0:128, 0:130],
                                    in_=ybufs[0][0][0:C, 0:128, :])
                nc.gpsimd.dma_start(out=out_d[:, 128:256, 0:130],
                                    in_=ybufs[1][0][0:C, 0:128, :])
            elif PARTS == "s2":
                for j in range(H // 8):
                    stage2_block(j)
            else:
                next_j = 0
                for sup in range(n_sup):
                    stage1_super(sup)
                    j_hi = min(4 * sup + 2, H // 8 - 1)
                    if sup == n_sup - 1:
                        j_hi = H // 8 - 1
                    while next_j <= j_hi:
                        stage2_block(next_j)
                        next_j += 1

    nc.compile()
    return nc


_MODULE_CACHE = {}


def _get_module():
    if "nc" not in _MODULE_CACHE:
        _MODULE_CACHE["nc"] = _build_module()
    return _MODULE_CACHE["nc"]


# ----------------------------------------------------------------------------
# Entry point
# ----------------------------------------------------------------------------

def prep_weight_map(pre_w, pre_b, post_w, post_b):
    """Device-layout weight arrays, shared across cores."""
    w1, b1 = _build_stage1_weights(np.asarray(pre_w), np.asarray(pre_b))
    w2 = _build_stage2_weights(np.asarray(post_w))
    b2 = np.asarray(post_b, np.float32).reshape(C, 1)

    w1_half = np.transpose(w1, (3, 1, 2, 0, 4)).reshape(C, 18 * 128)
    # w1_half[cin, ((ky*3+kx)*2+p)*128 + m] = w1[p, ky, kx, cin, m]
    w1_flat = np.ascontiguousarray(
        np.concatenate([w1_half, w1_half], axis=0)).astype(NP_BF16)
    w2_flat = np.ascontiguousarray(w2).astype(NP_BF16)
    return {
        "w1": w1_flat,
        "b1": np.ascontiguousarray(b1, np.float32),                # [128, 2]
        "w2": w2_flat,
        "b2": np.ascontiguousarray(np.vstack([b2, b2]), np.float32),
    }


def run(x, pre_w, pre_b, post_w, post_b, trace=False):
    x = np.asarray(x, np.float32)
    B = x.shape[0]
    assert B == N_CORES and x.shape == (B, C, H, W)

    wmap = prep_weight_map(pre_w, pre_b, post_w, post_b)
    x_bf = x.astype(NP_BF16)

    in_maps = []
    for b in range(B):
        in_maps.append({"x": np.ascontiguousarray(x_bf[b]), **wmap})

    nc = _get_module()
    res = run_bass_kernel_spmd(nc, in_maps, core_ids=list(range(N_CORES)),
                               trace=trace)
    out = np.stack([res.results[b]["out"] for b in range(B)])
    return out, res


def kernel(x, pre_w, pre_b, post_w, post_b):
    out, _ = run(x, pre_w, pre_b, post_w, post_b)
    return out
